# revision 36
# baseline (speedup 1.0000x reference)
"""Multi-head causal attention (B=1, S=4096, D=768, H=12) on 8 trn2 NeuronCores.

Sharding: tensor-parallel over heads + causal-balanced split of the query range.
  - cores 0-5 ("late"):  2 heads each, q in [1536, 4096), k in [0, 4096)
  - cores 6-7 ("early"): 6 heads each, q in [0, 1536),  k in [0, 1536)

Design (vs the fp32r baseline, 369us -> ~197us):
  - bf16 matmul operands everywhere; fp32 psum accumulation.  FWL (2x faster
    LDWEIGHTS) unlocks true row-tiled concurrency of the two heads' score
    matmuls (head A on PE rows 0-63, head B on rows 64-127, ~95ns/pair).
  - wavefront schedule: projection chunk-pairs are emitted just-in-time with
    two attention groups of lookahead, so the scalar engine's exp stream
    (the hard wall: 1 elem/lane/cycle @1.2GHz, ~116us/core) starts ~15us in
    and the PE stays dense enough to hold the HAM clock-gate at 8/8.
  - projection chunk-pairs share each stationary load (halved LDW tax); V is
    projected directly in [seq, dh] layout (stationary = x slice), killing
    the transpose stage and its psum-pool contention.
  - one exp instruction per (<=3-ktile x 2-head) group: [128, <=1536] free.
  - both heads' flash accumulators packed into ONE psum bank [65 used, 512]
    (start=True only on the first matmul: the bank-wide has_written clear
    makes head B's first accumulate an overwrite; stop=True only on the
    final matmul).  Out-projection psum shares the "av" pool tag and is
    emitted one qtile late so the in-order tensor queue never head-of-line
    blocks on the normalize chain.
  - normalize: dens -> sbuf, gpsimd partition_broadcast to 128 rows, then
    one reciprocal_approx_fast per (pair, qtile).  NOTE: the custom DVE op
    is only correct on full-height base-0 tiles on HW; [1,N]/psum inputs
    silently corrupt.
  - y written as bf16 partials, one strided DMA per qtile; host sums the
    core partials in fp32 and adds b_out.

All inputs are taken at full shape; slicing/packing happens on host.
"""

import sys
import threading

sys.path.insert(0, "/opt/trn_rl_repo")

import numpy as np
import ml_dtypes

import concourse.bass as bass
import concourse.mybir as mybir
import concourse.tile as tile
from concourse import bacc
from concourse.masks import make_identity

# ---------------------------------------------------------------- constants
B, S, D, H, DH = 1, 4096, 768, 12, 64
SCALE = DH ** -0.5
P = 128          # sbuf partitions
QT = 256         # query tile (free axis of scores)
KT = 128         # key tile (partition axis of scores)
CK = 256         # projection chunk (seq)
XW = 512         # x dma tile width (2 chunks)
GMAX = 3         # max ktiles per score/exp group
SPLIT = 1536     # early/late query split point
DT = mybir.dt.float32
BF = mybir.dt.bfloat16

CLASSES = {
    # name: (n_pairs, q0, q1, k_len)
    "late": (1, SPLIT, S, S),
    "early": (3, 0, SPLIT, SPLIT),
}


def _groups(n):
    """Split n (even) non-diagonal ktiles into chunks of 3 and 2."""
    out = []
    while n >= 5 or n == 3:
        out.append(3)
        n -= 3
    while n > 0:
        out.append(2)
        n -= 2
    return out


def build_module(cls):
    n_pairs, q0, q1, k_len = CLASSES[cls]
    f_c = P * n_pairs            # per-core feature width of each projection
    q_len = q1 - q0
    n_ck = k_len // CK           # projection chunks
    n_kt = k_len // KT           # ktiles of the core's k-support
    n_qt = q_len // QT           # qtiles of the core's q-range
    n_dt = D // P                # 6 contraction tiles for the projections
    c_q0 = q0 // CK              # first chunk whose q-projection is needed
    pre = (q0 + QT) // CK        # chunks needed before qtile 0 can run

    nc = bacc.Bacc("TRN2", target_bir_lowering=False, debug=False,
                   enable_asserts=True, num_devices=1)

    xT = nc.dram_tensor("xT", [D, k_len], BF, kind="ExternalInput")
    wqkvT = nc.dram_tensor("wqkvT", [P, 3 * n_dt * f_c], BF,
                           kind="ExternalInput")
    bq = nc.dram_tensor("bq", [f_c, 1], DT, kind="ExternalInput")
    bvb = nc.dram_tensor("bvb", [P, f_c], DT, kind="ExternalInput")
    woT = nc.dram_tensor("woT", [P, n_pairs * n_dt * P], BF,
                         kind="ExternalInput")
    dmask = nc.dram_tensor("dmask", [P, 2 * QT], BF, kind="ExternalInput")
    yT = nc.dram_tensor("yT", [D, q_len], BF, kind="ExternalOutput")

    with tile.TileContext(nc) as tc:
        with (
            tc.tile_pool(name="w", bufs=1) as sb_w,
            tc.tile_pool(name="x", bufs=4) as sb_x,
            tc.tile_pool(name="persist", bufs=1) as sb_per,
            tc.tile_pool(name="exp", bufs=5) as sb_exp,
            tc.tile_pool(name="aTp", bufs=3) as sb_a,
            tc.tile_pool(name="rn", bufs=3) as sb_rn,
            tc.tile_pool(name="yout", bufs=3) as sb_y,
            tc.tile_pool(name="big", bufs=2, space="PSUM") as ps_big,
            tc.tile_pool(name="av", bufs=2, space="PSUM") as ps_av,
        ):
            # ---------------- constants / weights to SBUF
            wqkv_sb = sb_w.tile([P, 3, n_dt, f_c], BF, tag="wqkv")
            nc.sync.dma_start(
                out=wqkv_sb,
                in_=wqkvT.rearrange("p (g t f) -> p g t f", g=3, t=n_dt))
            bq_sb = sb_w.tile([P, n_pairs], DT, tag="bq")
            nc.sync.dma_start(
                out=bq_sb, in_=bq.rearrange("(n p) o -> p (n o)", p=P))
            bvb_sb = sb_w.tile([P, f_c], DT, tag="bvb")
            nc.sync.dma_start(out=bvb_sb, in_=bvb.ap())
            wo_sb = sb_w.tile([P, n_pairs, n_dt, P], BF, tag="wo")
            nc.sync.dma_start(
                out=wo_sb,
                in_=woT.rearrange("p (n t m) -> p n t m", n=n_pairs, t=n_dt))
            dmask_sb = sb_w.tile([P, 2, QT], BF, tag="dmask")
            nc.sync.dma_start(
                out=dmask_sb, in_=dmask.rearrange("p (a q) -> p a q", a=2))
            # ---------------- PE warmup: ~6us of dummy matmuls during the
            # initial DMA wait so the HAM clock-gate reaches 8/8 before the
            # first projection chunk
            wup = sb_w.tile([P, QT], BF, tag="wup")
            nc.vector.memset(wup, 0.0)
            for _ in range(30):
                wps = ps_av.tile([P, 2, QT], DT, tag="av", name="wps")
                nc.tensor.matmul(wps[:, 0, :], wup[:, 0:128], wup,
                                 start=True, stop=True)
                nc.tensor.matmul(wps[:, 1, :], wup[:, 0:128], wup,
                                 start=True, stop=True)

            # ---------------- persistent activations (head pair packed on
            # partitions: head A rows 0-63, head B rows 64-127)
            qT = [sb_per.tile([P, q_len], BF, tag=f"qT{p}", name=f"qT{p}")
                  for p in range(n_pairs)]
            kT = [sb_per.tile([P, k_len], BF, tag=f"kT{p}", name=f"kT{p}")
                  for p in range(n_pairs)]
            # per ktile: [V_A(64) | 1 | pad | V_B(64) | 1 | pad], k on partitions
            vkt = [sb_per.tile([P, n_kt, 132], BF, tag=f"vk{p}", name=f"vk{p}")
                   for p in range(n_pairs)]
            for p in range(n_pairs):
                nc.vector.memset(vkt[p][:, :, 64:65], 1.0)
                nc.vector.memset(vkt[p][:, :, 130:131], 1.0)

            # ---------------- projection chunk (CK=256 seq positions)
            xmap = {}        # dma pair index -> xt tiles
            done_chunks = set()

            def pair_dma(pair):
                if pair not in xmap:
                    ps0 = pair * XW
                    w = min(XW, k_len - ps0)
                    xt = sb_x.tile([P, n_dt, XW], BF, tag="xt", name="xt")
                    nc.sync.dma_start(
                        out=xt[:, :, :w],
                        in_=xT.rearrange("(t p) s -> p t s", p=P)[
                            :, :, ps0:ps0 + w])
                    xmap[pair] = xt
                return xmap[pair]

            def emit_chunk(c):
                # emits the whole 2-chunk dma pair containing c: both chunks
                # share each stationary load (halves the LDWEIGHTS tax)
                pair = c // 2
                if pair in done_chunks or c >= n_ck:
                    return
                done_chunks.add(pair)
                ps0 = pair * XW
                w = min(XW, k_len - ps0)
                ncc = (w + CK - 1) // CK       # chunks in this pair (1 or 2)
                xt = pair_dma(pair)
                ccs = [cc for cc in range(ncc)]
                do_q = [ps0 + cc * CK + CK > q0 for cc in ccs]
                xts_cur = xt
                for p in range(n_pairs):
                    # psum [P, cc, qkv, CK]: the two concurrently-pending
                    # chains (same i, different cc) land in different banks;
                    # chains for different i never overlap (i-outer loop).
                    # v is projected directly in [seq, dh] layout (stationary
                    # = x slice), so no transpose stage is needed.
                    ps = ps_big.tile([P, 2, 3, CK], DT, tag="big",
                                     name="ps_prj")
                    for i in (0, 1):
                        for dti in range(n_dt):
                            for cc in ccs:
                                if i == 0 and not do_q[cc]:
                                    continue
                                nc.tensor.matmul(
                                    ps[:, cc, i, :],
                                    wqkv_sb[:, i, dti, p * P:(p + 1) * P],
                                    xts_cur[:, dti, cc * CK:(cc + 1) * CK],
                                    start=dti == 0, stop=dti == n_dt - 1)
                    for cc in ccs:
                        for j in range(CK // KT):
                            so = cc * CK + j * KT
                            for dti in range(n_dt):
                                nc.tensor.matmul(
                                    ps[:, cc, 2, j * KT:(j + 1) * KT],
                                    xts_cur[:, dti, so:so + KT],
                                    wqkv_sb[:, 2, dti, p * P:(p + 1) * P],
                                    start=dti == 0, stop=dti == n_dt - 1)
                    for cc in ccs:
                        s0 = ps0 + cc * CK
                        if n_pairs > 1:
                            nc.scalar.copy(
                                kT[p][:, s0:s0 + CK], ps[:, cc, 1, :])
                        else:
                            nc.vector.tensor_copy(
                                kT[p][:, s0:s0 + CK], ps[:, cc, 1, :])
                        if do_q[cc]:
                            lo = max(s0, q0)
                            nc.vector.tensor_scalar_add(
                                qT[p][:, lo - q0:s0 + CK - q0],
                                ps[:, cc, 0, lo - s0:CK], bq_sb[:, p:p + 1])
                        for j in range(CK // KT):
                            kt_i = (s0 // KT) + j
                            dst = vkt[p][:, kt_i, :].rearrange(
                                "p (h c) -> p h c", h=2)[:, :, 0:64]
                            nc.vector.tensor_add(
                                dst,
                                ps[:, cc, 2, j * KT:(j + 1) * KT].rearrange(
                                    "p (h c) -> p h c", h=2),
                                bvb_sb[:, p * P:(p + 1) * P].rearrange(
                                    "p (h c) -> p h c", h=2))

            # ---------------- group plans + chunk requirements
            def plan_for(qt):
                n_kt_q = 2 * (q0 // QT + qt) + 2
                return ([(c, False) for c in _groups(n_kt_q - 2)]
                        + [(2, True)], n_kt_q)

            # flattened (qt, gi) items with the chunks each one needs; the
            # emission loop ensures chunks a couple of groups ahead so the
            # projection pipeline overlaps the attention stream
            g_items = []     # (qt, gi) -> set of chunk indices
            for qt_ in range(n_qt):
                plan_, _ = plan_for(qt_)
                kt0_ = 0
                for gi_, (gsz_, _d) in enumerate(plan_):
                    req = {k // 2 for k in range(kt0_, kt0_ + gsz_)}
                    kt0_ += gsz_
                    if gi_ == 0 and not (qt_ == 0 and q0 > 0):
                        req.add((q0 + qt_ * QT) // CK)
                    g_items.append((qt_, gi_, req))
            g_base = {}
            for i, (qt_, gi_, _r) in enumerate(g_items):
                if gi_ == 0:
                    g_base[qt_] = i

            def ensure_ahead(gidx):
                for j in range(gidx, min(gidx + 3, len(g_items))):
                    for c in sorted(g_items[j][2]):
                        emit_chunk(c)

            # ---------------- attention qtile
            def emit_qtile(qt):
                plan, n_kt_q = plan_for(qt)
                a_tiles = []
                for p in range(n_pairs):
                    # av: one psum bank per (pair, qtile): head A numerator
                    # rows 0-63 + denominator row 64 in cols 0:256, head B in
                    # cols 256:512.  start=True only on the very first matmul
                    # (bank-wide has_written clear makes head B's first
                    # accumulate an overwrite), stop=True only on the last.
                    av = ps_av.tile([P, 2, QT], DT, tag="av", name="av")
                    qh = [qT[p][hi * 64:(hi + 1) * 64,
                                qt * QT:(qt + 1) * QT] for hi in (0, 1)]
                    kt0 = 0
                    for gi, (gsz, diag) in enumerate(plan):
                        kts = list(range(kt0, kt0 + gsz))
                        kt0 += gsz
                        if p == 0:
                            ensure_ahead(g_base[qt] + gi)
                        ps_sc = ps_big.tile([P, 2, GMAX, QT], DT, tag="big",
                                            name="ps_sc")
                        for j, k in enumerate(kts):
                            for hi in (0, 1):
                                nc.tensor.matmul(
                                    ps_sc[:, hi, j, :],
                                    kT[p][hi * 64:(hi + 1) * 64,
                                          k * KT:(k + 1) * KT],
                                    qh[hi], start=True, stop=True)
                        ex = sb_exp.tile([P, 2, GMAX, QT], BF, tag="ex")
                        nc.scalar.activation(
                            ex[:, :, 0:gsz, :], ps_sc[:, :, 0:gsz, :],
                            mybir.ActivationFunctionType.Exp, scale=SCALE)
                        if diag:
                            for hi in (0, 1):
                                nc.vector.tensor_mul(
                                    ex[:, hi, 0:2, :], ex[:, hi, 0:2, :],
                                    dmask_sb)
                        for j, k in enumerate(kts):
                            for hi in (0, 1):
                                nc.tensor.matmul(
                                    av[0:65, hi, :],
                                    vkt[p][:, k, 66 * hi:66 * hi + 65],
                                    ex[:, hi, j, :],
                                    start=(k == 0 and hi == 0),
                                    stop=(k == n_kt_q - 1 and hi == 1))
                    # normalize: a = num * (1/den)
                    aT = sb_a.tile([P, QT], BF, tag=f"aT{p}")
                    # dens: psum row 64 of each head's bank -> one sbuf row;
                    # broadcast raw dens to all partitions (gpsimd), then a
                    # full-tile reciprocal_approx_fast (the custom DVE op is
                    # only correct on [128, N] base-0 tiles on HW)
                    dd = sb_rn.tile([1, 2 * QT], DT, tag="dd")
                    nc.vector.tensor_copy(dd, av[64:65, :, :].rearrange(
                        "p h q -> p (h q)"))
                    db = sb_rn.tile([P, 2 * QT], DT, tag="db")
                    nc.gpsimd.partition_broadcast(db, dd)
                    rb = sb_rn.tile([P, 2, QT], DT, tag="rb")
                    nc.vector.reciprocal_approx_fast(
                        rb.rearrange("p h q -> p (h q)"), db)
                    for hi in (0, 1):
                        nc.vector.tensor_mul(
                            aT[hi * 64:(hi + 1) * 64, :],
                            av[0:64, hi, :], rb[hi * 64:hi * 64 + 64, hi, :])
                    a_tiles.append(aT)
                return a_tiles

            # out-projection (psum shares the "av" pool tag).  Emitted one
            # qtile late so the tensor queue never head-of-line blocks on
            # the normalize chain that produces a_tiles.
            def emit_outproj(qt, a_tiles):
                ysb = sb_y.tile([P, n_dt, QT], BF, tag="y")
                for mt in range(n_dt):
                    ps_y = ps_av.tile([P, 2, QT], DT, tag="av", name="ps_y")[:, 0, :]
                    for p in range(n_pairs):
                        nc.tensor.matmul(
                            ps_y, wo_sb[:, p, mt, :], a_tiles[p],
                            start=(p == 0), stop=(p == n_pairs - 1))
                    nc.vector.tensor_copy(ysb[:, mt, :], ps_y)
                nc.sync.dma_start(
                    out=yT.rearrange("(t p) q -> p t q", p=P)[
                        :, :, qt * QT:(qt + 1) * QT],
                    in_=ysb)

            # ---------------- schedule: wavefront (chunks just-in-time,
            # two attention groups of lookahead), out-projection one qtile
            # behind
            if q0 > 0:
                xt0 = pair_dma(c_q0 // 2)
                xo0 = (c_q0 % 2) * CK
                for p in range(n_pairs):
                    psq = ps_big.tile([P, 2, 3, CK], DT, tag="big",
                                      name="ps_qo")
                    for dti in range(n_dt):
                        nc.tensor.matmul(
                            psq[:, 0, 0, :],
                            wqkv_sb[:, 0, dti, p * P:(p + 1) * P],
                            xt0[:, dti, xo0:xo0 + CK],
                            start=dti == 0, stop=dti == n_dt - 1)
                    nc.vector.tensor_scalar_add(
                        qT[p][:, 0:CK], psq[:, 0, 0, :], bq_sb[:, p:p + 1])
            ensure_ahead(0)
            pending = None
            for qt in range(n_qt):
                a_tiles = emit_qtile(qt)
                if pending is not None:
                    emit_outproj(*pending)
                pending = (qt, a_tiles)
            emit_outproj(*pending)
            for c in range(n_ck):  # any chunk no qtile pulled (none expected)
                emit_chunk(c)

    nc.compile()
    return nc


# ---------------------------------------------------------------- host side
def _head_cols(heads):
    """column indices into a [*, 768] head-blocked axis for the given heads"""
    return np.concatenate([np.arange(h * DH, (h + 1) * DH) for h in heads])


def make_in_maps(x, W_in, b_in, W_out):
    """Returns (late_in_maps[6], early_in_maps[2])."""
    xT = np.ascontiguousarray(x.reshape(S, D).T).astype(ml_dtypes.bfloat16)
    WT = np.ascontiguousarray(W_in.T)                     # [768, 2304]
    WoT = np.ascontiguousarray(W_out.T)                   # [768, 768]

    tri = np.triu(np.ones((P, P), np.float32))            # k <= q
    dm = np.zeros((P, 2 * QT), np.float32)
    dm[:, 0:128] = tri          # diag ktile j=0: [tri | ones]
    dm[:, 128:256] = 1.0
    dm[:, 384:512] = tri        # diag ktile j=1: [zeros | tri]
    dm = dm.astype(ml_dtypes.bfloat16)

    def core_inputs(heads, cls):
        _, q0, q1, k_len = CLASSES[cls]
        cols = _head_cols(heads)
        bf = ml_dtypes.bfloat16
        wq = np.ascontiguousarray(WT[:, cols])
        wk = np.ascontiguousarray(WT[:, 768 + cols])
        wv = np.ascontiguousarray(WT[:, 1536 + cols])
        f_cc = len(cols)
        wqkv = np.concatenate([wq, wk, wv], axis=1)      # [768, 3*f_c]
        wqkv = (wqkv.reshape(6, 128, 3, f_cc).transpose(1, 2, 0, 3)
                .reshape(128, 18 * f_cc)).astype(bf)     # [p, (g, t, f)]
        bqc = np.ascontiguousarray(b_in[cols][:, None]).astype(np.float32)
        bvbc = np.ascontiguousarray(np.broadcast_to(
            b_in[1536 + cols][None, :], (P, len(cols)))).astype(np.float32)
        wo = WoT[cols, :]                                # [f_c, 768]
        wo = (wo.reshape(f_cc // 128, 128, 6, 128).transpose(1, 0, 2, 3)
              .reshape(128, -1)).astype(bf)
        return {
            "xT": np.ascontiguousarray(xT[:, :k_len]),
            "wqkvT": np.ascontiguousarray(wqkv),
            "bq": bqc, "bvb": bvbc, "woT": wo, "dmask": dm,
        }

    late = [core_inputs([2 * c, 2 * c + 1], "late") for c in range(6)]
    early = [core_inputs(list(range(6 * e, 6 * e + 6)), "early")
             for e in range(2)]
    return late, early


def assemble_output(late_res, early_res, b_out):
    yT = np.zeros((D, S), np.float32)
    for r in late_res:
        yT[:, SPLIT:] += np.asarray(r["yT"], dtype=np.float32)
    for r in early_res:
        yT[:, :SPLIT] += np.asarray(r["yT"], dtype=np.float32)
    y = yT.T + b_out[None, :]
    return y.reshape(B, S, D).astype(np.float32)


# ------------------------------------------------- pjrt runner (explicit devices)
def _run_group(nc, in_maps, devices):
    """run_bass_via_pjrt equivalent on an explicit device subset."""
    import jax
    from jax.sharding import Mesh, PartitionSpec
    from jax.experimental.shard_map import shard_map
    from concourse import bass2jax
    from concourse.bass2jax import _bass_exec_p, partition_id_tensor

    bass2jax.install_neuronx_cc_hook()
    n_cores = len(in_maps)
    partition_name = (nc.partition_id_tensor.name
                      if nc.partition_id_tensor else None)

    in_names, out_names, out_avals, zero_outs = [], [], [], []
    for alloc in nc.m.functions[0].allocations:
        if not isinstance(alloc, mybir.MemoryLocationSet):
            continue
        name = alloc.memorylocations[0].name
        if alloc.kind == "ExternalInput":
            if name != partition_name:
                in_names.append(name)
        elif alloc.kind == "ExternalOutput":
            shape = tuple(alloc.tensor_shape)
            dtype = mybir.dt.np(alloc.dtype)
            out_names.append(name)
            out_avals.append(jax.core.ShapedArray(shape, dtype))
            zero_outs.append(np.zeros(shape, dtype))
    n_params = len(in_names)
    n_outs = len(out_avals)
    in_names = in_names + out_names
    if partition_name is not None:
        in_names.append(partition_name)
    donate = tuple(range(n_params, n_params + n_outs))

    def _body(*args):
        operands = list(args)
        if partition_name is not None:
            operands.append(partition_id_tensor())
        outs = _bass_exec_p.bind(
            *operands,
            out_avals=tuple(out_avals),
            in_names=tuple(in_names),
            out_names=tuple(out_names),
            lowering_input_output_aliases=(),
            sim_require_finite=True,
            sim_require_nnan=True,
            nc=nc,
        )
        return tuple(outs)

    per_core = [[np.asarray(m[name]) for name in in_names[:n_params]]
                for m in in_maps]
    if n_cores == 1:
        out_arrs = jax.jit(_body, donate_argnums=donate, keep_unused=True)(
            *per_core[0], *zero_outs)
        return [{n: np.asarray(out_arrs[i]) for i, n in enumerate(out_names)}]

    mesh = Mesh(np.asarray(devices), ("core",))
    in_specs = (PartitionSpec("core"),) * (n_params + n_outs)
    out_specs = (PartitionSpec("core"),) * len(out_names)
    sharded = jax.jit(
        shard_map(_body, mesh=mesh, in_specs=in_specs, out_specs=out_specs,
                  check_rep=False),
        donate_argnums=donate, keep_unused=True)
    concat_in = [np.concatenate([per_core[c][i] for c in range(n_cores)],
                                axis=0) for i in range(n_params)]
    concat_zeros = [np.zeros((n_cores * z.shape[0], *z.shape[1:]), z.dtype)
                    for z in zero_outs]
    out_arrs = sharded(*concat_in, *concat_zeros)
    return [
        {n: np.asarray(out_arrs[i]).reshape(n_cores, *out_avals[i].shape)[c]
         for i, n in enumerate(out_names)}
        for c in range(n_cores)
    ]


_MODULES = {}
_WARM = set()


def _get_module(cls):
    if cls not in _MODULES:
        _MODULES[cls] = build_module(cls)
    return _MODULES[cls]


def kernel(x, W_in, b_in, W_out, b_out):
    import jax
    x = np.asarray(x, np.float32)
    W_in = np.asarray(W_in, np.float32)
    b_in = np.asarray(b_in, np.float32)
    W_out = np.asarray(W_out, np.float32)
    b_out = np.asarray(b_out, np.float32)

    late_maps, early_maps = make_in_maps(x, W_in, b_in, W_out)
    nc_late = _get_module("late")
    nc_early = _get_module("early")

    devs = jax.devices()
    results = {}
    errs = {}

    def run(tag, nc, maps, devices):
        try:
            results[tag] = _run_group(nc, maps, devices)
        except Exception as e:  # noqa: BLE001
            errs[tag] = e

    # first call per module compiles (serialize those); afterwards the two
    # device groups (cores 0-5 and 6-7) execute concurrently
    t1 = threading.Thread(target=run, args=("late", nc_late, late_maps, devs[0:6]))
    t2 = threading.Thread(target=run, args=("early", nc_early, early_maps, devs[6:8]))
    if not _WARM:
        t1.start(); t1.join()
        t2.start(); t2.join()
        _WARM.add(True)
    else:
        t1.start(); t2.start()
        t1.join(); t2.join()
    if errs:
        raise next(iter(errs.values()))

    return assemble_output(results["late"], results["early"], b_out)


# revision 38
# speedup vs baseline: 1.0649x; 1.0649x over previous
"""Multi-head causal attention (B=1, S=4096, D=768, H=12) on 8 trn2 NeuronCores.

Sharding: tensor-parallel over heads + causal-balanced split of the query range.
  - cores 0-5 ("late"):  2 heads each, q in [1536, 4096), k in [0, 4096)
  - cores 6-7 ("early"): 6 heads each, q in [0, 1536),  k in [0, 1536)

Design (vs the fp32r baseline, 369us -> ~197us):
  - bf16 matmul operands everywhere; fp32 psum accumulation.  FWL (2x faster
    LDWEIGHTS) unlocks true row-tiled concurrency of the two heads' score
    matmuls (head A on PE rows 0-63, head B on rows 64-127, ~95ns/pair).
  - wavefront schedule: projection chunk-pairs are emitted just-in-time with
    two attention groups of lookahead, so the scalar engine's exp stream
    (the hard wall: 1 elem/lane/cycle @1.2GHz, ~116us/core) starts ~15us in
    and the PE stays dense enough to hold the HAM clock-gate at 8/8.
  - projection chunk-pairs share each stationary load (halved LDW tax); V is
    projected directly in [seq, dh] layout (stationary = x slice), killing
    the transpose stage and its psum-pool contention.
  - one exp instruction per (<=3-ktile x 2-head) group: [128, <=1536] free.
  - both heads' flash accumulators packed into ONE psum bank [65 used, 512]
    (start=True only on the first matmul: the bank-wide has_written clear
    makes head B's first accumulate an overwrite; stop=True only on the
    final matmul).  Out-projection psum shares the "av" pool tag and is
    emitted one qtile late so the in-order tensor queue never head-of-line
    blocks on the normalize chain.
  - normalize: dens -> sbuf, gpsimd partition_broadcast to 128 rows, then
    one reciprocal_approx_fast per (pair, qtile).  NOTE: the custom DVE op
    is only correct on full-height base-0 tiles on HW; [1,N]/psum inputs
    silently corrupt.
  - y written as bf16 partials, one strided DMA per qtile; host sums the
    core partials in fp32 and adds b_out.

All inputs are taken at full shape; slicing/packing happens on host.
"""

import sys
import threading

sys.path.insert(0, "/opt/trn_rl_repo")

import numpy as np
import ml_dtypes

import concourse.bass as bass
import concourse.mybir as mybir
import concourse.tile as tile
from concourse import bacc
from concourse.masks import make_identity

# ---------------------------------------------------------------- constants
B, S, D, H, DH = 1, 4096, 768, 12, 64
SCALE = DH ** -0.5
P = 128          # sbuf partitions
QT = 256         # query tile (free axis of scores)
KT = 128         # key tile (partition axis of scores)
CK = 256         # projection chunk (seq)
XW = 512         # x dma tile width (2 chunks)
GMAX = 3         # max ktiles per score/exp group
SPLIT = 1536     # early/late query split point
DT = mybir.dt.float32
BF = mybir.dt.bfloat16

CLASSES = {
    # name: (n_pairs, q0, q1, k_len)
    "late": (1, SPLIT, S, S),
    "early": (3, 0, SPLIT, SPLIT),
}


def _groups(n):
    """Split n (even) non-diagonal ktiles into chunks of 3 and 2."""
    out = []
    while n >= 5 or n == 3:
        out.append(3)
        n -= 3
    while n > 0:
        out.append(2)
        n -= 2
    return out


def build_module(cls):
    n_pairs, q0, q1, k_len = CLASSES[cls]
    f_c = P * n_pairs            # per-core feature width of each projection
    q_len = q1 - q0
    n_ck = k_len // CK           # projection chunks
    n_kt = k_len // KT           # ktiles of the core's k-support
    n_qt = q_len // QT           # qtiles of the core's q-range
    n_dt = D // P                # 6 contraction tiles for the projections
    c_q0 = q0 // CK              # first chunk whose q-projection is needed
    pre = (q0 + QT) // CK        # chunks needed before qtile 0 can run

    nc = bacc.Bacc("TRN2", target_bir_lowering=False, debug=False,
                   enable_asserts=True, num_devices=1)

    xT = nc.dram_tensor("xT", [D, k_len], BF, kind="ExternalInput")
    wqkvT = nc.dram_tensor("wqkvT", [P, 3 * n_dt * f_c], BF,
                           kind="ExternalInput")
    bq = nc.dram_tensor("bq", [f_c, 1], DT, kind="ExternalInput")
    bvb = nc.dram_tensor("bvb", [P, f_c], DT, kind="ExternalInput")
    woT = nc.dram_tensor("woT", [P, n_pairs * n_dt * P], BF,
                         kind="ExternalInput")
    dmask = nc.dram_tensor("dmask", [P, 2 * QT], BF, kind="ExternalInput")
    yT = nc.dram_tensor("yT", [D, q_len], BF, kind="ExternalOutput")

    with tile.TileContext(nc) as tc:
        with (
            tc.tile_pool(name="w", bufs=1) as sb_w,
            tc.tile_pool(name="x", bufs=4) as sb_x,
            tc.tile_pool(name="persist", bufs=1) as sb_per,
            tc.tile_pool(name="exp", bufs=5) as sb_exp,
            tc.tile_pool(name="aTp", bufs=3) as sb_a,
            tc.tile_pool(name="rn", bufs=3) as sb_rn,
            tc.tile_pool(name="yout", bufs=3) as sb_y,
            tc.tile_pool(name="big", bufs=2, space="PSUM") as ps_big,
            tc.tile_pool(name="av", bufs=2, space="PSUM") as ps_av,
        ):
            # ---------------- constants / weights to SBUF
            wqkv_sb = sb_w.tile([P, 3, n_dt, f_c], BF, tag="wqkv")
            nc.sync.dma_start(
                out=wqkv_sb,
                in_=wqkvT.rearrange("p (g t f) -> p g t f", g=3, t=n_dt))
            bq_sb = sb_w.tile([P, n_pairs], DT, tag="bq")
            nc.sync.dma_start(
                out=bq_sb, in_=bq.rearrange("(n p) o -> p (n o)", p=P))
            bvb_sb = sb_w.tile([P, f_c], DT, tag="bvb")
            nc.sync.dma_start(out=bvb_sb, in_=bvb.ap())
            wo_sb = sb_w.tile([P, n_pairs, n_dt, P], BF, tag="wo")
            nc.sync.dma_start(
                out=wo_sb,
                in_=woT.rearrange("p (n t m) -> p n t m", n=n_pairs, t=n_dt))
            dmask_sb = sb_w.tile([P, 2, QT], BF, tag="dmask")
            nc.sync.dma_start(
                out=dmask_sb, in_=dmask.rearrange("p (a q) -> p a q", a=2))
            # ---------------- PE warmup: ~6us of dummy matmuls during the
            # initial DMA wait so the HAM clock-gate reaches 8/8 before the
            # first projection chunk
            wup = sb_w.tile([P, QT], BF, tag="wup")
            nc.vector.memset(wup, 0.0)
            for _ in range(30):
                wps = ps_av.tile([P, 2, QT], DT, tag="av", name="wps")
                nc.tensor.matmul(wps[:, 0, :], wup[:, 0:128], wup,
                                 start=True, stop=True)
                nc.tensor.matmul(wps[:, 1, :], wup[:, 0:128], wup,
                                 start=True, stop=True)

            # ---------------- persistent activations (head pair packed on
            # partitions: head A rows 0-63, head B rows 64-127)
            qT = [sb_per.tile([P, q_len], BF, tag=f"qT{p}", name=f"qT{p}")
                  for p in range(n_pairs)]
            kT = [sb_per.tile([P, k_len], BF, tag=f"kT{p}", name=f"kT{p}")
                  for p in range(n_pairs)]
            # per ktile: [V_A(64) | 1 | pad | V_B(64) | 1 | pad], k on partitions
            vkt = [sb_per.tile([P, n_kt, 132], BF, tag=f"vk{p}", name=f"vk{p}")
                   for p in range(n_pairs)]
            for p in range(n_pairs):
                nc.vector.memset(vkt[p][:, :, 64:65], 1.0)
                nc.vector.memset(vkt[p][:, :, 130:131], 1.0)

            # ---------------- projection chunk (CK=256 seq positions)
            xmap = {}        # dma pair index -> xt tiles
            done_chunks = set()

            def emit_chunk(c):
                # emits the whole 2-chunk dma pair containing c: both chunks
                # share each stationary load (halves the LDWEIGHTS tax)
                pair = c // 2
                if pair in done_chunks or c >= n_ck:
                    return
                done_chunks.add(pair)
                ps0 = pair * XW
                w = min(XW, k_len - ps0)
                ncc = (w + CK - 1) // CK       # chunks in this pair (1 or 2)
                xt = sb_x.tile([P, n_dt, XW], BF, tag="xt", name="xt")
                nc.sync.dma_start(
                    out=xt[:, :, :w],
                    in_=xT.rearrange("(t p) s -> p t s", p=P)[
                        :, :, ps0:ps0 + w])
                ccs = [cc for cc in range(ncc)]
                do_q = [ps0 + cc * CK + CK > q0 for cc in ccs]
                xts_cur = xt
                for p in range(n_pairs):
                    # psum [P, cc, qkv, CK]: the two concurrently-pending
                    # chains (same i, different cc) land in different banks;
                    # chains for different i never overlap (i-outer loop).
                    # v is projected directly in [seq, dh] layout (stationary
                    # = x slice), so no transpose stage is needed.
                    ps = ps_big.tile([P, 2, 3, CK], DT, tag="big",
                                     name="ps_prj")
                    for i in (0, 1):
                        for dti in range(n_dt):
                            for cc in ccs:
                                if i == 0 and not do_q[cc]:
                                    continue
                                nc.tensor.matmul(
                                    ps[:, cc, i, :],
                                    wqkv_sb[:, i, dti, p * P:(p + 1) * P],
                                    xts_cur[:, dti, cc * CK:(cc + 1) * CK],
                                    start=dti == 0, stop=dti == n_dt - 1)
                    for cc in ccs:
                        for j in range(CK // KT):
                            so = cc * CK + j * KT
                            for dti in range(n_dt):
                                nc.tensor.matmul(
                                    ps[:, cc, 2, j * KT:(j + 1) * KT],
                                    xts_cur[:, dti, so:so + KT],
                                    wqkv_sb[:, 2, dti, p * P:(p + 1) * P],
                                    start=dti == 0, stop=dti == n_dt - 1)
                    for cc in ccs:
                        s0 = ps0 + cc * CK
                        if n_pairs > 1:
                            nc.scalar.copy(
                                kT[p][:, s0:s0 + CK], ps[:, cc, 1, :])
                        else:
                            nc.vector.tensor_copy(
                                kT[p][:, s0:s0 + CK], ps[:, cc, 1, :])
                        if do_q[cc]:
                            lo = max(s0, q0)
                            nc.vector.tensor_scalar_add(
                                qT[p][:, lo - q0:s0 + CK - q0],
                                ps[:, cc, 0, lo - s0:CK], bq_sb[:, p:p + 1])
                        for j in range(CK // KT):
                            kt_i = (s0 // KT) + j
                            dst = vkt[p][:, kt_i, :].rearrange(
                                "p (h c) -> p h c", h=2)[:, :, 0:64]
                            nc.vector.tensor_add(
                                dst,
                                ps[:, cc, 2, j * KT:(j + 1) * KT].rearrange(
                                    "p (h c) -> p h c", h=2),
                                bvb_sb[:, p * P:(p + 1) * P].rearrange(
                                    "p (h c) -> p h c", h=2))

            # ---------------- group plans + chunk requirements
            def plan_for(qt):
                n_kt_q = 2 * (q0 // QT + qt) + 2
                return ([(c, False) for c in _groups(n_kt_q - 2)]
                        + [(2, True)], n_kt_q)

            # flattened (qt, gi) items with the chunks each one needs; the
            # emission loop ensures chunks a couple of groups ahead so the
            # projection pipeline overlaps the attention stream
            g_items = []     # (qt, gi) -> set of chunk indices
            for qt_ in range(n_qt):
                plan_, _ = plan_for(qt_)
                kt0_ = 0
                for gi_, (gsz_, _d) in enumerate(plan_):
                    req = {k // 2 for k in range(kt0_, kt0_ + gsz_)}
                    kt0_ += gsz_
                    if gi_ == 0:
                        req.add((q0 + qt_ * QT) // CK)
                    g_items.append((qt_, gi_, req))
            g_base = {}
            for i, (qt_, gi_, _r) in enumerate(g_items):
                if gi_ == 0:
                    g_base[qt_] = i

            def ensure_ahead(gidx):
                for j in range(gidx, min(gidx + 3, len(g_items))):
                    for c in sorted(g_items[j][2]):
                        emit_chunk(c)

            # ---------------- attention qtile
            def emit_qtile(qt):
                plan, n_kt_q = plan_for(qt)
                a_tiles = []
                for p in range(n_pairs):
                    # av: one psum bank per (pair, qtile): head A numerator
                    # rows 0-63 + denominator row 64 in cols 0:256, head B in
                    # cols 256:512.  start=True only on the very first matmul
                    # (bank-wide has_written clear makes head B's first
                    # accumulate an overwrite), stop=True only on the last.
                    av = ps_av.tile([P, 2, QT], DT, tag="av", name="av")
                    qh = [qT[p][hi * 64:(hi + 1) * 64,
                                qt * QT:(qt + 1) * QT] for hi in (0, 1)]
                    kt0 = 0
                    pend_av = None   # (kts, ex) of the previous group

                    def emit_av(kts_, ex_):
                        for j, k in enumerate(kts_):
                            for hi in (0, 1):
                                nc.tensor.matmul(
                                    av[0:65, hi, :],
                                    vkt[p][:, k, 66 * hi:66 * hi + 65],
                                    ex_[:, hi, j, :],
                                    start=(k == 0 and hi == 0),
                                    stop=(k == n_kt_q - 1 and hi == 1))

                    # software-pipelined: group g's AV is emitted after group
                    # g+1's scores/exp, so the in-order tensor queue never
                    # head-of-line blocks on the exp the AV depends on
                    for gi, (gsz, diag) in enumerate(plan):
                        kts = list(range(kt0, kt0 + gsz))
                        kt0 += gsz
                        if p == 0:
                            ensure_ahead(g_base[qt] + gi)
                        ps_sc = ps_big.tile([P, 2, GMAX, QT], DT, tag="big",
                                            name="ps_sc")
                        for j, k in enumerate(kts):
                            for hi in (0, 1):
                                nc.tensor.matmul(
                                    ps_sc[:, hi, j, :],
                                    kT[p][hi * 64:(hi + 1) * 64,
                                          k * KT:(k + 1) * KT],
                                    qh[hi], start=True, stop=True)
                        ex = sb_exp.tile([P, 2, GMAX, QT], BF, tag="ex")
                        nc.scalar.activation(
                            ex[:, :, 0:gsz, :], ps_sc[:, :, 0:gsz, :],
                            mybir.ActivationFunctionType.Exp, scale=SCALE)
                        if diag:
                            for hi in (0, 1):
                                nc.vector.tensor_mul(
                                    ex[:, hi, 0:2, :], ex[:, hi, 0:2, :],
                                    dmask_sb)
                        if pend_av is not None:
                            emit_av(*pend_av)
                        pend_av = (kts, ex)
                    emit_av(*pend_av)
                    # normalize: a = num * (1/den)
                    aT = sb_a.tile([P, QT], BF, tag=f"aT{p}")
                    # dens: psum row 64 of each head's bank -> one sbuf row;
                    # broadcast raw dens to all partitions (gpsimd), then a
                    # full-tile reciprocal_approx_fast (the custom DVE op is
                    # only correct on [128, N] base-0 tiles on HW)
                    dd = sb_rn.tile([1, 2 * QT], DT, tag="dd")
                    nc.vector.tensor_copy(dd, av[64:65, :, :].rearrange(
                        "p h q -> p (h q)"))
                    db = sb_rn.tile([P, 2 * QT], DT, tag="db")
                    nc.gpsimd.partition_broadcast(db, dd)
                    rb = sb_rn.tile([P, 2, QT], DT, tag="rb")
                    nc.vector.reciprocal_approx_fast(
                        rb.rearrange("p h q -> p (h q)"), db)
                    for hi in (0, 1):
                        nc.vector.tensor_mul(
                            aT[hi * 64:(hi + 1) * 64, :],
                            av[0:64, hi, :], rb[hi * 64:hi * 64 + 64, hi, :])
                    a_tiles.append(aT)
                return a_tiles

            # out-projection (psum shares the "av" pool tag).  Emitted one
            # qtile late so the tensor queue never head-of-line blocks on
            # the normalize chain that produces a_tiles.
            def emit_outproj(qt, a_tiles):
                ysb = sb_y.tile([P, n_dt, QT], BF, tag="y")
                for mt in range(n_dt):
                    ps_y = ps_av.tile([P, 2, QT], DT, tag="av", name="ps_y")[:, 0, :]
                    for p in range(n_pairs):
                        nc.tensor.matmul(
                            ps_y, wo_sb[:, p, mt, :], a_tiles[p],
                            start=(p == 0), stop=(p == n_pairs - 1))
                    nc.vector.tensor_copy(ysb[:, mt, :], ps_y)
                nc.sync.dma_start(
                    out=yT.rearrange("(t p) q -> p t q", p=P)[
                        :, :, qt * QT:(qt + 1) * QT],
                    in_=ysb)

            # ---------------- schedule: wavefront (chunks just-in-time,
            # two attention groups of lookahead), out-projection one qtile
            # behind
            ensure_ahead(0)
            pending = None
            for qt in range(n_qt):
                a_tiles = emit_qtile(qt)
                if pending is not None:
                    emit_outproj(*pending)
                pending = (qt, a_tiles)
            emit_outproj(*pending)
            for c in range(n_ck):  # any chunk no qtile pulled (none expected)
                emit_chunk(c)

    nc.compile()
    return nc


# ---------------------------------------------------------------- host side
def _head_cols(heads):
    """column indices into a [*, 768] head-blocked axis for the given heads"""
    return np.concatenate([np.arange(h * DH, (h + 1) * DH) for h in heads])


def make_in_maps(x, W_in, b_in, W_out):
    """Returns (late_in_maps[6], early_in_maps[2])."""
    xT = np.ascontiguousarray(x.reshape(S, D).T).astype(ml_dtypes.bfloat16)
    WT = np.ascontiguousarray(W_in.T)                     # [768, 2304]
    WoT = np.ascontiguousarray(W_out.T)                   # [768, 768]

    tri = np.triu(np.ones((P, P), np.float32))            # k <= q
    dm = np.zeros((P, 2 * QT), np.float32)
    dm[:, 0:128] = tri          # diag ktile j=0: [tri | ones]
    dm[:, 128:256] = 1.0
    dm[:, 384:512] = tri        # diag ktile j=1: [zeros | tri]
    dm = dm.astype(ml_dtypes.bfloat16)

    def core_inputs(heads, cls):
        _, q0, q1, k_len = CLASSES[cls]
        cols = _head_cols(heads)
        bf = ml_dtypes.bfloat16
        wq = np.ascontiguousarray(WT[:, cols])
        wk = np.ascontiguousarray(WT[:, 768 + cols])
        wv = np.ascontiguousarray(WT[:, 1536 + cols])
        f_cc = len(cols)
        wqkv = np.concatenate([wq, wk, wv], axis=1)      # [768, 3*f_c]
        wqkv = (wqkv.reshape(6, 128, 3, f_cc).transpose(1, 2, 0, 3)
                .reshape(128, 18 * f_cc)).astype(bf)     # [p, (g, t, f)]
        bqc = np.ascontiguousarray(b_in[cols][:, None]).astype(np.float32)
        bvbc = np.ascontiguousarray(np.broadcast_to(
            b_in[1536 + cols][None, :], (P, len(cols)))).astype(np.float32)
        wo = WoT[cols, :]                                # [f_c, 768]
        wo = (wo.reshape(f_cc // 128, 128, 6, 128).transpose(1, 0, 2, 3)
              .reshape(128, -1)).astype(bf)
        return {
            "xT": np.ascontiguousarray(xT[:, :k_len]),
            "wqkvT": np.ascontiguousarray(wqkv),
            "bq": bqc, "bvb": bvbc, "woT": wo, "dmask": dm,
        }

    late = [core_inputs([2 * c, 2 * c + 1], "late") for c in range(6)]
    early = [core_inputs(list(range(6 * e, 6 * e + 6)), "early")
             for e in range(2)]
    return late, early


def assemble_output(late_res, early_res, b_out):
    yT = np.zeros((D, S), np.float32)
    for r in late_res:
        yT[:, SPLIT:] += np.asarray(r["yT"], dtype=np.float32)
    for r in early_res:
        yT[:, :SPLIT] += np.asarray(r["yT"], dtype=np.float32)
    y = yT.T + b_out[None, :]
    return y.reshape(B, S, D).astype(np.float32)


# ------------------------------------------------- pjrt runner (explicit devices)
def _run_group(nc, in_maps, devices):
    """run_bass_via_pjrt equivalent on an explicit device subset."""
    import jax
    from jax.sharding import Mesh, PartitionSpec
    from jax.experimental.shard_map import shard_map
    from concourse import bass2jax
    from concourse.bass2jax import _bass_exec_p, partition_id_tensor

    bass2jax.install_neuronx_cc_hook()
    n_cores = len(in_maps)
    partition_name = (nc.partition_id_tensor.name
                      if nc.partition_id_tensor else None)

    in_names, out_names, out_avals, zero_outs = [], [], [], []
    for alloc in nc.m.functions[0].allocations:
        if not isinstance(alloc, mybir.MemoryLocationSet):
            continue
        name = alloc.memorylocations[0].name
        if alloc.kind == "ExternalInput":
            if name != partition_name:
                in_names.append(name)
        elif alloc.kind == "ExternalOutput":
            shape = tuple(alloc.tensor_shape)
            dtype = mybir.dt.np(alloc.dtype)
            out_names.append(name)
            out_avals.append(jax.core.ShapedArray(shape, dtype))
            zero_outs.append(np.zeros(shape, dtype))
    n_params = len(in_names)
    n_outs = len(out_avals)
    in_names = in_names + out_names
    if partition_name is not None:
        in_names.append(partition_name)
    donate = tuple(range(n_params, n_params + n_outs))

    def _body(*args):
        operands = list(args)
        if partition_name is not None:
            operands.append(partition_id_tensor())
        outs = _bass_exec_p.bind(
            *operands,
            out_avals=tuple(out_avals),
            in_names=tuple(in_names),
            out_names=tuple(out_names),
            lowering_input_output_aliases=(),
            sim_require_finite=True,
            sim_require_nnan=True,
            nc=nc,
        )
        return tuple(outs)

    per_core = [[np.asarray(m[name]) for name in in_names[:n_params]]
                for m in in_maps]
    if n_cores == 1:
        out_arrs = jax.jit(_body, donate_argnums=donate, keep_unused=True)(
            *per_core[0], *zero_outs)
        return [{n: np.asarray(out_arrs[i]) for i, n in enumerate(out_names)}]

    mesh = Mesh(np.asarray(devices), ("core",))
    in_specs = (PartitionSpec("core"),) * (n_params + n_outs)
    out_specs = (PartitionSpec("core"),) * len(out_names)
    sharded = jax.jit(
        shard_map(_body, mesh=mesh, in_specs=in_specs, out_specs=out_specs,
                  check_rep=False),
        donate_argnums=donate, keep_unused=True)
    concat_in = [np.concatenate([per_core[c][i] for c in range(n_cores)],
                                axis=0) for i in range(n_params)]
    concat_zeros = [np.zeros((n_cores * z.shape[0], *z.shape[1:]), z.dtype)
                    for z in zero_outs]
    out_arrs = sharded(*concat_in, *concat_zeros)
    return [
        {n: np.asarray(out_arrs[i]).reshape(n_cores, *out_avals[i].shape)[c]
         for i, n in enumerate(out_names)}
        for c in range(n_cores)
    ]


_MODULES = {}
_WARM = set()


def _get_module(cls):
    if cls not in _MODULES:
        _MODULES[cls] = build_module(cls)
    return _MODULES[cls]


def kernel(x, W_in, b_in, W_out, b_out):
    import jax
    x = np.asarray(x, np.float32)
    W_in = np.asarray(W_in, np.float32)
    b_in = np.asarray(b_in, np.float32)
    W_out = np.asarray(W_out, np.float32)
    b_out = np.asarray(b_out, np.float32)

    late_maps, early_maps = make_in_maps(x, W_in, b_in, W_out)
    nc_late = _get_module("late")
    nc_early = _get_module("early")

    devs = jax.devices()
    results = {}
    errs = {}

    def run(tag, nc, maps, devices):
        try:
            results[tag] = _run_group(nc, maps, devices)
        except Exception as e:  # noqa: BLE001
            errs[tag] = e

    # first call per module compiles (serialize those); afterwards the two
    # device groups (cores 0-5 and 6-7) execute concurrently
    t1 = threading.Thread(target=run, args=("late", nc_late, late_maps, devs[0:6]))
    t2 = threading.Thread(target=run, args=("early", nc_early, early_maps, devs[6:8]))
    if not _WARM:
        t1.start(); t1.join()
        t2.start(); t2.join()
        _WARM.add(True)
    else:
        t1.start(); t2.start()
        t1.join(); t2.join()
    if errs:
        raise next(iter(errs.values()))

    return assemble_output(results["late"], results["early"], b_out)
